# revision 1
# baseline (speedup 1.0000x reference)
"""Trainium2 Bass kernel for nn_NetworkLayer_79173427134941 (gnn_message_passing).

Reference computation (per batch item b, N=1024 points, 3D coords):
    norms = ||x_b||                      [N, 1]
    dots  = sqrt(x_b @ x_b^T)            [N, N]
    scalars = [u_b (G=8) | norms | dots] [N, 1033]
    h = LeakyReLU(scalars @ W0 + b0); h = LeakyReLU(h @ W1 + b1)
    fk = h @ W2 + b2                     [N, 128]
    out_b = einsum('io,id->od', fk, x_b) / N    [128, 3]

Strategy:
  - Data-parallel over batch: 4 batch items per core x 8 cores.
  - Never materialize dots in HBM: gram on TensorE (fp32r), sqrt on ScalarE
    during PSUM->SBUF eviction, MLP fused on-chip in transposed [H, N] layout.
  - u-part + b0 folded into a host-precomputed K=2 rhs chunk [norms; ones].
  - Final contraction uses associativity:
       out_b^T = (x_b^T @ h1) @ W2 + b2 (x) colsum(x_b)
    so the device only returns y_b = x_b^T @ h1  [3, 128]; the last tiny
    [3,128]@[128,128] matmul + bias outer product runs on host.
"""

import numpy as np

B, N, G = 32, 1024, 8
H, K_OUT = 128, 128
N_CORES = 8
BPC = B // N_CORES  # batch items per core

_cached = {}
# "f32r": all matmuls fp32r (max accuracy). "f16": dots + MLP tail in fp16
# (1 cyc/row at any moving size + fast weight loads, ~5e-4 quantization).
PRECISION = "f16"
# PE row-tiling of the gram was tried and abandoned: the row-group matmuls
# (32x128 array mode) interleave with 128-row MLP matmuls, and the required
# array-mode-switch drains are not emitted by this toolchain (fp32r weights
# even fault the exec unit; fp16 silently corrupts the odd strips).
ROWTILE_GRAM = False



def _build_nc(precision=None, repeat=1, with_b1=True):
    import concourse.tile as tile
    from concourse import bacc, mybir

    precision = precision or PRECISION
    f32 = mybir.dt.float32
    f32r = mybir.dt.float32r
    f16 = mybir.dt.float16
    tdt = f16 if precision == "f16" else f32r   # tail: h0/h1c/w1/xc
    mdt = f16 if precision == "f16" else f32r   # mid: dots/w0d
    MUL = mybir.AluOpType.mult
    ADD = mybir.AluOpType.add

    nc = bacc.Bacc(
        "TRN2",
        target_bir_lowering=False,
        debug=False,
        enable_asserts=True,
        num_devices=N_CORES,
    )

    # DRAM I/O (per core)
    gdt = tdt  # gram operand dtype (fp16: fast weight loads, no fp32 self-load)
    xT_d = nc.dram_tensor("xT", [BPC, 3, N], gdt, kind="ExternalInput").ap()
    xbc_d = nc.dram_tensor("xbc", [BPC, 128, 3 * N], tdt, kind="ExternalInput").ap()
    rhs2_d = nc.dram_tensor("rhs2", [BPC, 2, N], tdt, kind="ExternalInput").ap()
    lw2_d = nc.dram_tensor("lw2", [BPC, 2, H], tdt, kind="ExternalInput").ap()
    w0d_d = nc.dram_tensor("w0d", [128, 1024], mdt, kind="ExternalInput").ap()
    w1_d = nc.dram_tensor("w1", [128, H], tdt, kind="ExternalInput").ap()
    b1t_d = ones_d = None
    if with_b1:
        b1t_d = nc.dram_tensor("b1t", [1, N], tdt, kind="ExternalInput").ap()
        ones_d = nc.dram_tensor("ones", [1, N], tdt, kind="ExternalInput").ap()
    y_d = nc.dram_tensor("y", [BPC, H, 3], f32, kind="ExternalOutput").ap()

    NCHUNK = N // 128  # 8 K-chunks of the dots matmul

    with tile.TileContext(nc) as tc:
        with (
            tc.tile_pool(name="const", bufs=1) as constp,
            tc.tile_pool(name="data", bufs=2) as datap,
            tc.tile_pool(name="dots", bufs=2) as dotsp,
            tc.tile_pool(name="act", bufs=2) as actp,
            tc.tile_pool(name="yout", bufs=2) as youtp,
            tc.tile_pool(name="gram", bufs=2, space="PSUM") as gramp,
            tc.tile_pool(name="h0p", bufs=1, space="PSUM") as h0pp,
            tc.tile_pool(name="h1p", bufs=1, space="PSUM") as h1pp,
        ):
            # const tiles (DMAs issued after batch-0 loads; see emit_consts)
            w0d_sb = constp.tile([128, 1024], mdt)
            w1_sb = constp.tile([128, H], tdt)
            b1t_sb = ones_sb = None
            if with_b1:
                b1t_sb = constp.tile([1, N], tdt, name="b1t_sb")
                ones_sb = constp.tile([1, N], tdt, name="ones_sb")

            def emit_consts():
                nc.sync.dma_start(out=w0d_sb[:], in_=w0d_d[:])
                nc.sync.dma_start(out=w1_sb[:], in_=w1_d[:])
                if with_b1:
                    nc.sync.dma_start(out=b1t_sb[:], in_=b1t_d[:])
                    nc.sync.dma_start(out=ones_sb[:], in_=ones_d[:])

            def leaky_evict(out_ap, ps_ap, tmp_ap, use_act=False):
                # leaky(x) = 0.01*x + 0.99*relu(x); two ops so each reads PSUM once.
                # use_act routes the relu-scale half to ScalarE (only worth it
                # for the last batch item, when the sqrt stream has drained).
                if use_act:
                    nc.scalar.activation(
                        tmp_ap, ps_ap, mybir.ActivationFunctionType.Relu,
                        bias=0.0, scale=0.99,
                    )
                else:
                    nc.vector.tensor_scalar(
                        tmp_ap, ps_ap, 0.0, 0.99, mybir.AluOpType.max, MUL
                    )
                nc.vector.scalar_tensor_tensor(out_ap, ps_ap, 0.01, tmp_ap, MUL, ADD)

            def emit_gram_strip(b, m, st):
                """Gram strip m of batch b: 2 matmuls + sqrt eviction."""
                if m == 0:
                    xt_parts = 35 if ROWTILE_GRAM else 3
                    xT_sb = datap.tile([xt_parts, N], gdt, tag="xT", name=f"xT{b}")
                    nc.sync.dma_start(out=xT_sb[0:3, :], in_=xT_d[b])
                    if ROWTILE_GRAM:
                        nc.sync.dma_start(out=xT_sb[32:35, :], in_=xT_d[b])
                    rhs2_sb = datap.tile([2, N], tdt, tag="rhs2", name=f"rhs2{b}")
                    nc.sync.dma_start(out=rhs2_sb[:], in_=rhs2_d[b])
                    lw2_sb = datap.tile([2, H], tdt, tag="lw2", name=f"lw2{b}")
                    nc.sync.dma_start(out=lw2_sb[:], in_=lw2_d[b])
                    if b == 0:
                        emit_consts()
                    dots_sb = dotsp.tile([128, NCHUNK * N], mdt, tag="dots",
                                         name=f"dots{b}")
                    # x^T rows pre-broadcast across partitions on the host;
                    # consumed by the DVE y-reduction
                    xbc_sb = datap.tile([128, 3 * N], tdt, tag="xbc",
                                        name=f"xbc{b}")
                    nc.sync.dma_start(out=xbc_sb[:], in_=xbc_d[b])
                    st.update(xT=xT_sb, xbc=xbc_sb, rhs2=rhs2_sb, lw2=lw2_sb,
                              dots=dots_sb)
                xT_sb, dots_sb = st["xT"], st["dots"]
                g_ps = gramp.tile([128, N], f32, tag="g", name=f"g{b}_{m}")
                lhsT = xT_sb[:, 128 * m : 128 * (m + 1)]
                for half in range(2):
                    nc.tensor.matmul(
                        g_ps[:, 512 * half : 512 * (half + 1)],
                        lhsT,
                        xT_sb[:, 512 * half : 512 * (half + 1)],
                        start=True,
                        stop=True,
                    )
                nc.scalar.sqrt(dots_sb[:, N * m : N * (m + 1)], g_ps[:])

            def emit_h0_chunk(b, c, st):
                """Layer-0 K-chunk c of batch b (needs dots strip c only)."""
                if c == 0:
                    h0_ps = h0pp.tile([128, N], f32, tag="h0ps", name=f"h0ps{b}")
                    st["h0ps"] = h0_ps
                    for half in range(2):
                        sl = slice(512 * half, 512 * (half + 1))
                        nc.tensor.matmul(
                            h0_ps[:, sl],
                            st["lw2"][:],
                            st["rhs2"][:, sl],
                            start=True,
                            stop=False,
                        )
                h0_ps, dots_sb = st["h0ps"], st["dots"]
                lhsT = w0d_sb[:, 128 * c : 128 * (c + 1)]
                for half in range(2):
                    nc.tensor.matmul(
                        h0_ps[:, 512 * half : 512 * (half + 1)],
                        lhsT,
                        dots_sb[:, N * c + 512 * half : N * c + 512 * (half + 1)],
                        start=False,
                        stop=(c == NCHUNK - 1),
                    )
                if c == NCHUNK - 1:
                    h0_sb = actp.tile([128, N], tdt, tag="h0", name=f"h0{b}")
                    st["h0"] = h0_sb
                    for half in range(2):
                        sl = slice(512 * half, 512 * (half + 1))
                        ltmp = actp.tile([128, 512], f32, tag="ltmp", bufs=4,
                                         name=f"ltmp0_{b}_{half}")
                        leaky_evict(h0_sb[:, sl], h0_ps[:, sl], ltmp[:],
                                    use_act=(b == BPC - 1))

            def emit_tail(b, st):
                """Layer 1 (transposed [H, N] layout) + output contraction."""
                h0_sb, xbc_sb = st["h0"], st["xbc"]
                h1_ps = h1pp.tile([128, N], f32, tag="h1ps", name=f"h1ps{b}")
                for half in range(2):
                    sl = slice(512 * half, 512 * (half + 1))
                    if with_b1:
                        # bias as a rank-1 matmul b1 (x) ones; skipped when
                        # the host sees b1 == 0 (true for this problem)
                        nc.tensor.matmul(
                            h1_ps[:, sl],
                            b1t_sb[:, 0:128],
                            ones_sb[:, sl],
                            start=True,
                            stop=False,
                        )
                    nc.tensor.matmul(
                        h1_ps[:, sl],
                        w1_sb[:],
                        h0_sb[:, sl],
                        start=not with_b1,
                        stop=True,
                    )
                h1c_sb = actp.tile([128, N], tdt, tag="h1c", name=f"h1c{b}")
                for half in range(2):
                    sl = slice(512 * half, 512 * (half + 1))
                    ltmp1 = actp.tile([128, 512], f32, tag="ltmp", bufs=4,
                                      name=f"ltmp1_{b}_{half}")
                    leaky_evict(h1c_sb[:, sl], h1_ps[:, sl], ltmp1[:],
                                use_act=(b == BPC - 1))

                # y_b^T[h, d] = sum_i h1^T[h, i] * x[i, d]: free-axis
                # multiply-reduce on DVE against the broadcast x rows
                yT_sb = youtp.tile([128, 4], f32, tag="y", name=f"y{b}")
                for d in range(3):
                    ysc = actp.tile([128, N], tdt, tag="ysc", name=f"ysc{b}_{d}")
                    nc.vector.scalar_tensor_tensor(
                        ysc[:],
                        h1c_sb[:],
                        1.0,
                        xbc_sb[:, N * d : N * (d + 1)],
                        MUL,
                        MUL,
                        accum_out=yT_sb[:, d : d + 1],
                    )
                nc.sync.dma_start(out=y_d[b], in_=yT_sb[:, 0:3])

            # Software-pipelined emission, one stage per batch item:
            #   [gram strips b] [tail of b-1] [h0 chunks of b]
            # Priorities follow emission order, so the previous item's
            # MLP tail fills TensorE while ScalarE streams this item's
            # sqrts; h0 chunk c only needs sqrt strip c, so the h0 block
            # drains right behind the sqrt stream.
            def emit_all():
                states = [dict() for _ in range(BPC)]
                for b in range(BPC):
                    for m in range(NCHUNK):
                        emit_gram_strip(b, m, states[b])
                    if b >= 1:
                        emit_tail(b - 1, states[b - 1])
                    for c in range(NCHUNK):
                        emit_h0_chunk(b, c, states[b])
                emit_tail(BPC - 1, states[BPC - 1])

            if repeat == 1:
                emit_all()
            else:
                # benchmark mode: repeat the whole (idempotent) pipeline so
                # device time dominates host/tunnel dispatch overhead
                with tc.For_i(0, repeat, 1):
                    emit_all()

    nc.finalize()
    return nc


def _host_prep(x, u, W0, b0, W1, b1):
    """Build per-core input maps."""
    tnp = np.float16 if PRECISION == "f16" else np.float32
    gnp = tnp
    xT = np.ascontiguousarray(x.transpose(0, 2, 1)).astype(gnp)  # [B, 3, N]
    # [B, 128, 3N]: row d of x^T broadcast across the partition dim
    xbc = np.ascontiguousarray(
        np.broadcast_to(xT.reshape(B, 1, 3 * N), (B, 128, 3 * N))
    )
    norms = np.sqrt((x.astype(np.float64) ** 2).sum(-1)).astype(np.float32)  # [B, N]
    rhs2 = np.stack([norms, np.ones_like(norms)], axis=1)  # [B, 2, N]
    cb = (u @ W0[:G] + b0).astype(np.float32)  # [B, H]
    w0n = np.broadcast_to(W0[G], (B, H)).astype(np.float32)
    lw2 = np.ascontiguousarray(np.stack([w0n, cb], axis=1))  # [B, 2, H]
    w0d = np.ascontiguousarray(
        W0[G + 1 :].reshape(N // 128, 128, H).transpose(1, 0, 2).reshape(128, N // 128 * H)
    )

    in_maps = []
    for c in range(N_CORES):
        sl = slice(BPC * c, BPC * (c + 1))
        in_maps.append(
            {
                "xT": np.ascontiguousarray(xT[sl]),
                "xbc": np.ascontiguousarray(xbc[sl]),
                "rhs2": np.ascontiguousarray(rhs2[sl]).astype(tnp),
                "lw2": np.ascontiguousarray(lw2[sl]).astype(tnp),
                "w0d": w0d.astype(tnp),
                "w1": np.ascontiguousarray(W1).astype(tnp),
                "b1t": np.tile(b1, N // H)[None, :].astype(tnp),
                "ones": np.ones((1, N), dtype=tnp),
            }
        )
    return in_maps


def kernel(x, u, W0, b0, W1, b1, W2, b2, _run_kwargs=None):
    x = np.asarray(x, dtype=np.float32)
    u = np.asarray(u, dtype=np.float32)
    W0 = np.asarray(W0, dtype=np.float32)
    b0 = np.asarray(b0, dtype=np.float32)
    W1 = np.asarray(W1, dtype=np.float32)
    b1 = np.asarray(b1, dtype=np.float32)
    W2 = np.asarray(W2, dtype=np.float32)
    b2 = np.asarray(b2, dtype=np.float32)

    from concourse.bass_utils import run_bass_kernel_spmd

    with_b1 = bool(np.any(b1))
    key = ("nc", with_b1)
    if key not in _cached:
        _cached[key] = _build_nc(with_b1=with_b1)
    nc = _cached[key]

    in_maps = _host_prep(x, u, W0, b0, W1, b1)
    kw = dict(_run_kwargs or {})
    res = run_bass_kernel_spmd(nc, in_maps, list(range(N_CORES)), **kw)
    _cached["last_results"] = res
    y = np.concatenate([r["y"] for r in res.results], axis=0)  # [B, H, 3]

    # host finish: out[b,o,d] = sum_h W2[h,o] y[b,h,d] / N + b2[o]*colsum_x[b,d]/N
    colsum = x.sum(axis=1)  # [B, 3]
    out = (
        np.einsum("ho,bhd->bod", W2.astype(np.float64), y.astype(np.float64))
        + b2.astype(np.float64)[None, :, None] * colsum.astype(np.float64)[:, None, :]
    ) / N
    return out.astype(np.float32)



# revision 8
# speedup vs baseline: 2.3414x; 2.3414x over previous
"""Trainium2 Bass kernel for nn_NetworkLayer_79173427134941 (gnn_message_passing).

Reference computation (per batch item b, N=1024 points, 3D coords):
    norms = ||x_b||                      [N, 1]
    dots  = sqrt(x_b @ x_b^T)            [N, N]
    scalars = [u_b (G=8) | norms | dots] [N, 1033]
    h = LeakyReLU(scalars @ W0 + b0); h = LeakyReLU(h @ W1 + b1)
    fk = h @ W2 + b2                     [N, 128]
    out_b = einsum('io,id->od', fk, x_b) / N    [128, 3]

Strategy (v2):
  - Data-parallel over batch: 4 batch items per core x 8 cores.
  - Gram on TensorE in fp16 ([3,N] lhsT slices); sqrt evicted by ScalarE
    directly to fp8e4 with a power-of-two rescale folded into the
    activation scale (dots/S stored, W0d*S on host; S cancels in h0).
  - Layer-0 matmul in fp8 DoubleRow perf mode (0.5 cyc/row): 4 k-pair
    matmuls per 512-wide half instead of 8 fp16 matmuls.
  - fp8 W0d quantization error is mean-compensated on the host:
    cb += mu_dots * colsum(W0d - w0d8/S), mu_dots estimated by sampling.
  - LeakyReLU evictions split across engines: Pool does
    tmp = max(x,0)*0.99, DVE does out = 0.01*x + tmp (one op each).
  - Layer-1 runs with SWAPPED operand roles (lhsT = h0c tile, rhs = W1),
    which produces h1 in natural [i, h] block layout for free; the final
    contraction y = h1^T x then runs as 8 tiny PE matmuls against
    natural-layout x chunks. This removes the 3 MB/core broadcast-x HBM
    stream and the DVE y-reduction of v1 entirely.
  - Device returns y_b = x_b^T @ h1 [H, 3]; the last [3,128]@[128,128]
    matmul + bias outer product run on the host (W2, b2 folded there).
"""

import numpy as np
import ml_dtypes

B, N, G = 32, 1024, 8
H, K_OUT = 128, 128
N_CORES = 8
BPC = B // N_CORES  # batch items per core
NSTRIP = N // 128   # 8 gram strips / layer-0 k-chunks

DOTS_SCALE = 4.0    # dots stored as fp8(dots/S); W0d stored as fp8(W0d*S)

_cached = {}


def _register_leaky():
    """Custom 1-op DVE leaky-relu: out = max(s0*x, x). Registered at runtime
    (the stock scalar_tensor_tensor path needs 2 ops and the dup-operand
    max form fails walrus codegen)."""
    from concourse import dve_ops
    from concourse.dve_spec import Spec, Src0, C0, maxx, lower
    from concourse.dve_uop import DveOpSpec

    for op in dve_ops.OPS:
        if op.name == "LEAKY_ANT":
            return op
    spec = Spec(
        body=maxx(Src0 * C0, Src0),
        reference=lambda in0, in1, s0, s1, imm2: np.maximum(in0 * s0, in0),
    )
    shas = {}
    for ver in ("v3", "v4"):
        tmp = DveOpSpec(name="LEAKY_ANT", opcode=1,
                        uops=lower(spec, ver=ver), rd1_en=False)
        shas[ver] = tmp.sha(ver)
    op = dve_ops.DveOp("LEAKY_ANT", spec, subdim=False, uops_sha=shas)
    dve_ops.OPS.append(op)
    dve_ops.CUSTOM_DVE_SPECS[op.name] = spec
    dve_ops._SUB_OPCODE_FOR_NAME[op.name] = (
        dve_ops._CUSTOM_DVE_ROW_BASE + len(dve_ops.OPS) - 1
    )
    assert dve_ops._SUB_OPCODE_FOR_NAME[op.name] < 0x20
    return op


def _build_nc(repeat=1, with_b1=True):
    import concourse.tile as tile
    from concourse import bacc, mybir

    f32 = mybir.dt.float32
    f16 = mybir.dt.float16
    f8 = mybir.dt.float8e4
    MUL = mybir.AluOpType.mult
    ADD = mybir.AluOpType.add
    MAX = mybir.AluOpType.max
    DR = mybir.MatmulPerfMode.DoubleRow
    SQRT = mybir.ActivationFunctionType.Sqrt

    LEAKY = _register_leaky()
    nc = bacc.Bacc(
        "TRN2",
        target_bir_lowering=False,
        debug=False,
        enable_asserts=True,
        num_devices=N_CORES,
    )

    # DRAM I/O (per core)
    xT_d = nc.dram_tensor("xT", [BPC, 3, N], f16, kind="ExternalInput").ap()
    xn_d = nc.dram_tensor("xn", [BPC, 128, 3 * NSTRIP], f16, kind="ExternalInput").ap()
    rhs2_d = nc.dram_tensor("rhs2", [BPC, 2, N], f16, kind="ExternalInput").ap()
    lw2_d = nc.dram_tensor("lw2", [BPC, 2, H], f16, kind="ExternalInput").ap()
    w0d_d = nc.dram_tensor("w0d", [128, NSTRIP * H], f8, kind="ExternalInput").ap()
    w1_d = nc.dram_tensor("w1", [128, H], f16, kind="ExternalInput").ap()
    b1t_d = ones1_d = None
    if with_b1:
        b1t_d = nc.dram_tensor("b1t", [1, N], f16, kind="ExternalInput").ap()
        ones1_d = nc.dram_tensor("ones1", [1, 128], f16, kind="ExternalInput").ap()
    y_d = nc.dram_tensor("y", [BPC, H, 3], f32, kind="ExternalOutput").ap()

    SC = 1.0 / (DOTS_SCALE * DOTS_SCALE)  # sqrt(g*SC) = dots/S

    with tile.TileContext(nc) as tc:
        with (
            tc.tile_pool(name="const", bufs=1) as constp,
            tc.tile_pool(name="data", bufs=2) as datap,
            tc.tile_pool(name="dots", bufs=2) as dotsp,
            tc.tile_pool(name="act", bufs=2) as actp,
            tc.tile_pool(name="yout", bufs=2) as youtp,
            tc.tile_pool(name="gram", bufs=2, space="PSUM") as gramp,
            tc.tile_pool(name="h0p", bufs=1, space="PSUM") as h0pp,
            tc.tile_pool(name="h1p", bufs=1, space="PSUM") as h1pp,
        ):
            w0d_sb = constp.tile([128, NSTRIP * H], f8)
            w1_sb = constp.tile([128, H], f16)
            b1t_sb = ones1_sb = None
            if with_b1:
                b1t_sb = constp.tile([1, N], f16, name="b1t_sb")
                ones1_sb = constp.tile([1, 128], f16, name="ones1_sb")

            def emit_consts():
                nc.sync.dma_start(out=w0d_sb[:], in_=w0d_d[:])
                nc.sync.dma_start(out=w1_sb[:], in_=w1_d[:])
                if with_b1:
                    nc.sync.dma_start(out=b1t_sb[:], in_=b1t_d[:])
                    nc.sync.dma_start(out=ones1_sb[:], in_=ones1_d[:])

            def leaky_evict(out_ap, ps_ap):
                # 1-op custom DVE leaky: out = max(0.01*x, x)
                nc.vector._custom_dve(LEAKY, out=out_ap, in0=ps_ap, s0=0.01)

            def emit_gram_strip(b, m, st):
                """Gram strip m of batch b: 2 fp16 matmuls + fp8 sqrt evict."""
                if m == 0:
                    xT_sb = datap.tile([3, N], f16, tag="xT", name=f"xT{b}")
                    nc.sync.dma_start(out=xT_sb[:], in_=xT_d[b])
                    rhs2_sb = datap.tile([2, N], f16, tag="rhs2", name=f"rhs2{b}")
                    nc.sync.dma_start(out=rhs2_sb[:], in_=rhs2_d[b])
                    lw2_sb = datap.tile([2, H], f16, tag="lw2", name=f"lw2{b}")
                    nc.sync.dma_start(out=lw2_sb[:], in_=lw2_d[b])
                    xn_sb = datap.tile([128, 3 * NSTRIP], f16, tag="xn",
                                       name=f"xn{b}")
                    nc.sync.dma_start(out=xn_sb[:], in_=xn_d[b])
                    if b == 0:
                        emit_consts()
                    dots_sb = dotsp.tile([128, NSTRIP * N], f8, tag="dots",
                                         name=f"dots{b}")
                    st.update(xT=xT_sb, xn=xn_sb, rhs2=rhs2_sb, lw2=lw2_sb,
                              dots=dots_sb)
                xT_sb, dots_sb = st["xT"], st["dots"]
                g_ps = gramp.tile([128, N], f32, tag="g", name=f"g{b}_{m}")
                lhsT = xT_sb[:, 128 * m: 128 * (m + 1)]
                for half in range(2):
                    nc.tensor.matmul(
                        g_ps[:, 512 * half: 512 * (half + 1)],
                        lhsT,
                        xT_sb[:, 512 * half: 512 * (half + 1)],
                        start=True,
                        stop=True,
                    )
                nc.scalar.activation(
                    dots_sb[:, N * m: N * (m + 1)], g_ps[:], SQRT,
                    bias=0.0, scale=SC,
                )

            def emit_h0_init(b, st):
                """Layer-0 psum init: [W0 norm-row | folded bias] @ rhs2."""
                h0_ps = h0pp.tile([128, N], f32, tag="h0ps", name=f"h0ps{b}")
                st["h0ps"] = h0_ps
                for half in range(2):
                    sl = slice(512 * half, 512 * (half + 1))
                    nc.tensor.matmul(
                        h0_ps[:, sl], st["lw2"][:], st["rhs2"][:, sl],
                        start=True, stop=False,
                    )

            def emit_h0_pair(b, c, st):
                """fp8 DoubleRow k-pair matmul over dots strips (2c, 2c+1)."""
                h0_ps, dots_sb = st["h0ps"], st["dots"]
                lhsT = w0d_sb[:, 256 * c: 256 * (c + 1)].rearrange(
                    "p (t n) -> p t n", t=2)
                pair = dots_sb[:, 2048 * c: 2048 * (c + 1)].rearrange(
                    "p (t h n) -> p t h n", t=2, h=2)
                for half in range(2):
                    nc.tensor.matmul(
                        h0_ps[:, 512 * half: 512 * (half + 1)],
                        lhsT,
                        pair[:, :, half, :],
                        start=False,
                        stop=(c == NSTRIP // 2 - 1),
                        perf_mode=DR,
                    )

            def emit_h0_leaky(b, st):
                h0c_sb = actp.tile([128, N], f16, tag="h0c", name=f"h0c{b}")
                leaky_evict(h0c_sb[:], st["h0ps"][:])
                st["h0c"] = h0c_sb

            def emit_tail(b, st):
                """Layer 1 (swapped roles -> natural layout) + y matmuls."""
                h0c_sb, xn_sb = st["h0c"], st["xn"]
                h1_ps = h1pp.tile([128, N], f32, tag="h1ps", name=f"h1ps{b}")
                if with_b1:
                    nc.tensor.matmul(
                        h1_ps[:], ones1_sb[:], b1t_sb[:],
                        start=True, stop=False,
                    )
                for t in range(NSTRIP):
                    sl = slice(128 * t, 128 * (t + 1))
                    nc.tensor.matmul(
                        h1_ps[:, sl],
                        h0c_sb[:, sl],
                        w1_sb[:],
                        start=not with_b1,
                        stop=True,
                    )
                h1n_sb = actp.tile([128, N], f16, tag="h1n", name=f"h1n{b}")
                leaky_evict(h1n_sb[:], h1_ps[:])

                # y accumulator rotates through the h0 PSUM ring (free here:
                # item b's h0 was fully evicted before its tail runs)
                y_ps = h0pp.tile([128, N], f32, tag="h0ps", name=f"yps{b}")
                for t in range(NSTRIP):
                    nc.tensor.matmul(
                        y_ps[:, 0:3],
                        h1n_sb[:, 128 * t: 128 * (t + 1)],
                        xn_sb[:, 3 * t: 3 * (t + 1)],
                        start=(t == 0),
                        stop=(t == NSTRIP - 1),
                    )
                y_sb = youtp.tile([128, 4], f32, tag="y", name=f"y{b}")
                nc.vector.tensor_copy(y_sb[:, 0:3], y_ps[:, 0:3])
                nc.sync.dma_start(out=y_d[b], in_=y_sb[:, 0:3])

            # Software-pipelined emission. Priorities follow emission order,
            # so h0 pair-matmuls are slotted between gram strips: the PE
            # queue then has ready work while ScalarE streams the sqrts
            # (the Act engine is the bottleneck; PE must never make it wait).
            def emit_all():
                states = [dict() for _ in range(BPC)]
                for b in range(BPC):
                    st = states[b]
                    for m in range(4):
                        emit_gram_strip(b, m, st)
                    emit_h0_init(b, st)
                    for c in range(3):
                        emit_gram_strip(b, 4 + c, st)
                        emit_h0_pair(b, c, st)
                    emit_gram_strip(b, 7, st)
                    if b >= 1:
                        emit_tail(b - 1, states[b - 1])
                    emit_h0_pair(b, 3, st)
                    emit_h0_leaky(b, st)
                emit_tail(BPC - 1, states[BPC - 1])

            if repeat == 1:
                emit_all()
            else:
                with tc.For_i(0, repeat, 1):
                    emit_all()

    nc.finalize()
    return nc


def _host_prep(x, u, W0, b0, W1, b1):
    """Build per-core input maps."""
    f8 = ml_dtypes.float8_e4m3
    S = DOTS_SCALE
    xT = np.ascontiguousarray(x.transpose(0, 2, 1)).astype(np.float16)  # [B,3,N]
    xn = np.ascontiguousarray(
        x.reshape(B, NSTRIP, 128, 3).transpose(0, 2, 1, 3).reshape(B, 128, 3 * NSTRIP)
    ).astype(np.float16)
    norms = np.sqrt((x.astype(np.float64) ** 2).sum(-1)).astype(np.float32)  # [B,N]
    rhs2 = np.stack([norms, np.ones_like(norms)], axis=1).astype(np.float16)

    W0d = W0[G + 1:]                                    # [N, H]
    w0d8 = (W0d * S).astype(f8)                         # stored fp8
    w0d8_pe = np.ascontiguousarray(
        w0d8.reshape(NSTRIP, 128, H).transpose(1, 0, 2).reshape(128, NSTRIP * H)
    )
    # mean-field compensation for W0d quantization, using a sampled mu_dots
    rng = np.random.default_rng(12345)
    ii = rng.integers(0, N, size=8192)
    jj = rng.integers(0, N, size=8192)
    mu = np.sqrt(np.maximum((x[:, ii, :] * x[:, jj, :]).sum(-1), 0.0)).mean(axis=1)
    dW = W0d - w0d8.astype(np.float32) / S              # [N, H]
    cb = (u @ W0[:G] + b0).astype(np.float32)           # [B, H]
    cb = cb + mu[:, None] * dW.sum(axis=0)[None, :]
    w0n = np.broadcast_to(W0[G], (B, H)).astype(np.float32)
    lw2 = np.ascontiguousarray(np.stack([w0n, cb], axis=1)).astype(np.float16)

    in_maps = []
    for c in range(N_CORES):
        sl = slice(BPC * c, BPC * (c + 1))
        in_maps.append(
            {
                "xT": np.ascontiguousarray(xT[sl]),
                "xn": np.ascontiguousarray(xn[sl]),
                "rhs2": np.ascontiguousarray(rhs2[sl]),
                "lw2": np.ascontiguousarray(lw2[sl]),
                "w0d": w0d8_pe,
                "w1": np.ascontiguousarray(W1).astype(np.float16),
                "b1t": np.tile(b1, N // H)[None, :].astype(np.float16),
                "ones1": np.ones((1, 128), dtype=np.float16),
            }
        )
    return in_maps


def kernel(x, u, W0, b0, W1, b1, W2, b2, _run_kwargs=None):
    x = np.asarray(x, dtype=np.float32)
    u = np.asarray(u, dtype=np.float32)
    W0 = np.asarray(W0, dtype=np.float32)
    b0 = np.asarray(b0, dtype=np.float32)
    W1 = np.asarray(W1, dtype=np.float32)
    b1 = np.asarray(b1, dtype=np.float32)
    W2 = np.asarray(W2, dtype=np.float32)
    b2 = np.asarray(b2, dtype=np.float32)

    from concourse.bass_utils import run_bass_kernel_spmd

    with_b1 = bool(np.any(b1))
    key = ("nc", with_b1)
    if key not in _cached:
        _cached[key] = _build_nc(with_b1=with_b1)
    nc = _cached[key]

    in_maps = _host_prep(x, u, W0, b0, W1, b1)
    kw = dict(_run_kwargs or {})
    res = run_bass_kernel_spmd(nc, in_maps, list(range(N_CORES)), **kw)
    _cached["last_results"] = res
    y = np.concatenate([r["y"] for r in res.results], axis=0)  # [B, H, 3]

    # host finish: out[b,o,d] = sum_h W2[h,o] y[b,h,d] / N + b2[o]*colsum_x[b,d]/N
    colsum = x.sum(axis=1)  # [B, 3]
    out = (
        np.einsum("ho,bhd->bod", W2.astype(np.float64), y.astype(np.float64))
        + b2.astype(np.float64)[None, :, None] * colsum.astype(np.float64)[:, None, :]
    ) / N
    return out.astype(np.float32)


# revision 20
# speedup vs baseline: 2.5626x; 1.0945x over previous
"""Trainium2 Bass kernel for nn_NetworkLayer_79173427134941 (gnn_message_passing).

Reference computation (per batch item b, N=1024 points, 3D coords):
    norms = ||x_b||                      [N, 1]
    dots  = sqrt(x_b @ x_b^T)            [N, N]
    scalars = [u_b (G=8) | norms | dots] [N, 1033]
    h = LeakyReLU(scalars @ W0 + b0); h = LeakyReLU(h @ W1 + b1)
    fk = h @ W2 + b2                     [N, 128]
    out_b = einsum('io,id->od', fk, x_b) / N    [128, 3]

Strategy (v2):
  - Data-parallel over batch: 4 batch items per core x 8 cores.
  - Gram on TensorE in fp16 ([3,N] lhsT slices); sqrt evicted by ScalarE
    directly to fp8e4 with a power-of-two rescale folded into the
    activation scale (dots/S stored, W0d*S on host; S cancels in h0).
  - Layer-0 matmul in fp8 DoubleRow perf mode (0.5 cyc/row): 4 k-pair
    matmuls per 512-wide half instead of 8 fp16 matmuls.
  - fp8 W0d quantization error is mean-compensated on the host:
    cb += mu_dots * colsum(W0d - w0d8/S), mu_dots estimated by sampling.
  - LeakyReLU evictions split across engines: Pool does
    tmp = max(x,0)*0.99, DVE does out = 0.01*x + tmp (one op each).
  - Layer-1 runs with SWAPPED operand roles (lhsT = h0c tile, rhs = W1),
    which produces h1 in natural [i, h] block layout for free; the final
    contraction y = h1^T x then runs as 8 tiny PE matmuls against
    natural-layout x chunks. This removes the 3 MB/core broadcast-x HBM
    stream and the DVE y-reduction of v1 entirely.
  - Device returns y_b = x_b^T @ h1 [H, 3]; the last [3,128]@[128,128]
    matmul + bias outer product run on the host (W2, b2 folded there).
"""

import numpy as np
import ml_dtypes

B, N, G = 32, 1024, 8
H, K_OUT = 128, 128
N_CORES = 8
BPC = B // N_CORES  # batch items per core
NSTRIP = N // 128   # 8 gram strips / layer-0 k-chunks

DOTS_SCALE = 4.0    # dots stored as fp8(dots/S); W0d stored as fp8(W0d*S)


_cached = {}


def _register_leaky():
    """Custom 1-op DVE leaky-relu: out = max(s0*x, x). Registered at runtime
    (the stock scalar_tensor_tensor path needs 2 ops and the dup-operand
    max form fails walrus codegen)."""
    from concourse import dve_ops
    from concourse.dve_spec import Spec, Src0, C0, maxx, lower
    from concourse.dve_uop import DveOpSpec

    for op in dve_ops.OPS:
        if op.name == "LEAKY_ANT":
            return op
    spec = Spec(
        body=maxx(Src0 * C0, Src0),
        reference=lambda in0, in1, s0, s1, imm2: np.maximum(in0 * s0, in0),
    )
    shas = {}
    for ver in ("v3", "v4"):
        tmp = DveOpSpec(name="LEAKY_ANT", opcode=1,
                        uops=lower(spec, ver=ver), rd1_en=False)
        shas[ver] = tmp.sha(ver)
    op = dve_ops.DveOp("LEAKY_ANT", spec, subdim=False, uops_sha=shas)
    dve_ops.OPS.append(op)
    dve_ops.CUSTOM_DVE_SPECS[op.name] = spec
    dve_ops._SUB_OPCODE_FOR_NAME[op.name] = (
        dve_ops._CUSTOM_DVE_ROW_BASE + len(dve_ops.OPS) - 1
    )
    assert dve_ops._SUB_OPCODE_FOR_NAME[op.name] < 0x20
    return op


def _build_nc(repeat=1, with_b1=True):
    import concourse.tile as tile
    from concourse import bacc, mybir

    f32 = mybir.dt.float32
    f16 = mybir.dt.float16
    f8 = mybir.dt.float8e4
    MUL = mybir.AluOpType.mult
    ADD = mybir.AluOpType.add
    MAX = mybir.AluOpType.max
    DR = mybir.MatmulPerfMode.DoubleRow
    SQRT = mybir.ActivationFunctionType.Sqrt
    PRELU = mybir.ActivationFunctionType.Prelu

    LEAKY = _register_leaky()
    nc = bacc.Bacc(
        "TRN2",
        target_bir_lowering=False,
        debug=False,
        enable_asserts=True,
        num_devices=N_CORES,
    )

    # DRAM I/O (per core)
    xT_d = nc.dram_tensor("xT", [BPC, 3, N], f16, kind="ExternalInput").ap()
    xn_d = nc.dram_tensor("xn", [BPC, 128, 3 * NSTRIP], f16, kind="ExternalInput").ap()
    rhs2_d = nc.dram_tensor("rhs2", [BPC, 2, N], f16, kind="ExternalInput").ap()
    lw2_d = nc.dram_tensor("lw2", [BPC, 2, H], f16, kind="ExternalInput").ap()
    w0d_d = nc.dram_tensor("w0d", [128, NSTRIP * H], f8, kind="ExternalInput").ap()
    w1_d = nc.dram_tensor("w1", [128, H], f16, kind="ExternalInput").ap()
    ident_d = nc.dram_tensor("ident", [128, 128], f8, kind="ExternalInput").ap()
    b1t_d = ones1_d = None
    if with_b1:
        b1t_d = nc.dram_tensor("b1t", [1, N], f16, kind="ExternalInput").ap()
        ones1_d = nc.dram_tensor("ones1", [1, 128], f16, kind="ExternalInput").ap()
    y_d = nc.dram_tensor("y", [BPC, H, 3], f32, kind="ExternalOutput").ap()

    SC = 1.0 / (DOTS_SCALE * DOTS_SCALE)  # sqrt(g*SC) = dots/S

    with tile.TileContext(nc) as tc:
        with (
            tc.tile_pool(name="const", bufs=1) as constp,
            tc.tile_pool(name="data", bufs=2) as datap,
            tc.tile_pool(name="dots", bufs=2) as dotsp,
            tc.tile_pool(name="act", bufs=2) as actp,
            tc.tile_pool(name="yout", bufs=2) as youtp,
            tc.tile_pool(name="gram", bufs=2, space="PSUM") as gramp,
            tc.tile_pool(name="h0p", bufs=1, space="PSUM") as h0pp,
            tc.tile_pool(name="h1p", bufs=1, space="PSUM") as h1pp,
            tc.tile_pool(name="slab", bufs=1, space="PSUM") as slabp,
        ):
            w0d_sb = constp.tile([128, NSTRIP * H], f8)
            w1_sb = constp.tile([128, H], f16)
            ident_sb = constp.tile([128, 128], f8)
            b1t_sb = ones1_sb = None
            if with_b1:
                b1t_sb = constp.tile([1, N], f16, name="b1t_sb")
                ones1_sb = constp.tile([1, 128], f16, name="ones1_sb")

            def emit_consts():
                nc.sync.dma_start(out=w0d_sb[:], in_=w0d_d[:])
                nc.sync.dma_start(out=w1_sb[:], in_=w1_d[:])
                nc.sync.dma_start(out=ident_sb[:], in_=ident_d[:])
                if with_b1:
                    nc.sync.dma_start(out=b1t_sb[:], in_=b1t_d[:])
                    nc.sync.dma_start(out=ones1_sb[:], in_=ones1_d[:])

            def leaky_evict(outA, outB, ps_ap, on_act=False):
                # leaky halves into two separate half-tiles (separate tiles
                # so the writes never serialize on tile-level dep tracking).
                # on_act (pipeline drain): second half runs on ScalarE as
                # Prelu -- same act table as Sqrt, no reload.
                nc.vector._custom_dve(
                    LEAKY, out=outA[:], in0=ps_ap[:, 0:512], s0=0.01)
                if on_act:
                    nc.scalar.activation(
                        outB[:], ps_ap[:, 512:], PRELU,
                        bias=0.0, scale=1.0, alpha=0.01)
                else:
                    nc.vector._custom_dve(
                        LEAKY, out=outB[:], in0=ps_ap[:, 512:], s0=0.01)

            def emit_gram_strip(b, m, st):
                """Gram strip m + fp8 sqrt (half-symmetric scheme).

                Strips 0-3 compute full rows dots[128m:128(m+1), :]; strips
                4-7 compute only columns [512, 128(m+1)). Their left halves
                (columns [0, 512)) are mirrors of strips 0-3's right-half
                tiles and are reconstructed by PE identity-transposes.
                """
                if m == 0:
                    xT_sb = datap.tile([3, N], f16, tag="xT", name=f"xT{b}")
                    nc.sync.dma_start(out=xT_sb[:], in_=xT_d[b])
                    rhs2_sb = datap.tile([2, N], f16, tag="rhs2", name=f"rhs2{b}")
                    nc.sync.dma_start(out=rhs2_sb[:], in_=rhs2_d[b])
                    lw2_sb = datap.tile([2, H], f16, tag="lw2", name=f"lw2{b}")
                    nc.sync.dma_start(out=lw2_sb[:], in_=lw2_d[b])
                    xn_sb = datap.tile([128, 3 * NSTRIP], f16, tag="xn",
                                       name=f"xn{b}")
                    nc.sync.dma_start(out=xn_sb[:], in_=xn_d[b])
                    if b == 0:
                        emit_consts()
                    dots_sb = dotsp.tile([128, NSTRIP * N], f8, tag="dots",
                                         name=f"dots{b}")
                    st.update(xT=xT_sb, xn=xn_sb, rhs2=rhs2_sb, lw2=lw2_sb,
                              dots=dots_sb)
                xT_sb, dots_sb = st["xT"], st["dots"]
                lhsT = xT_sb[:, 128 * m: 128 * (m + 1)]
                g_ps = gramp.tile([128, N], f32, tag="g", name=f"g{b}_{m}")
                for half in range(2):
                    nc.tensor.matmul(
                        g_ps[:, 512 * half: 512 * (half + 1)],
                        lhsT,
                        xT_sb[:, 512 * half: 512 * (half + 1)],
                        start=True,
                        stop=True,
                    )
                nc.scalar.activation(
                    dots_sb[:, N * m: N * (m + 1)], g_ps[:], SQRT,
                    bias=0.0, scale=SC,
                )

            def emit_gram_rights(b, m, st):
                """Right halves (cols 512:1024) of strips m and m+1 (m in
                {4, 6}) share one PSUM tile and one sqrt instruction."""
                xT_sb, dots_sb = st["xT"], st["dots"]
                g_ps = gramp.tile([128, N], f32, tag="g", name=f"gr{b}_{m}")
                for i in range(2):
                    nc.tensor.matmul(
                        g_ps[:, 512 * i: 512 * (i + 1)],
                        xT_sb[:, 128 * (m + i): 128 * (m + i + 1)],
                        xT_sb[:, 512:1024],
                        start=True,
                        stop=True,
                    )
                dst = dots_sb[:].rearrange("p (s n) -> p s n", s=NSTRIP)[
                    :, m: m + 2, 512:1024]
                nc.scalar.activation(dst, g_ps[:], SQRT, bias=0.0, scale=SC)

            def emit_slab(b, m, st):
                """PE identity-transposes: strip m's (m>=4) left-half tiles
                are mirrors of strips 0-3's block-m tiles. One [128,512]
                slab per strip from a dedicated 1-bank pool, so no gram or
                h1 ring dependency ever blocks the transpose chain."""
                dots_sb = st["dots"]
                slab = slabp.tile([128, 512], f32, tag="slab", name=f"s{b}_{m}")
                for p in range(4):
                    nc.tensor.matmul(
                        slab[:, 128 * p: 128 * (p + 1)],
                        dots_sb[:, N * p + 128 * m: N * p + 128 * (m + 1)],
                        ident_sb[:],
                        start=True,
                        stop=True,
                    )
                st[f"slab{m}"] = slab

            def emit_evict(b, m, st):
                """Copy slab m (fp32 PSUM) into dots strip m columns [0,512)."""
                dots_sb, slab = st["dots"], st[f"slab{m}"]
                nc.vector.tensor_copy(
                    dots_sb[:, N * m: N * m + 512], slab[:])

            def emit_h0_init(b, st):
                """Layer-0 psum init: [W0 norm-row | folded bias] @ rhs2."""
                h0_ps = h0pp.tile([128, N], f32, tag="h0ps", name=f"h0ps{b}")
                st["h0ps"] = h0_ps
                for half in range(2):
                    sl = slice(512 * half, 512 * (half + 1))
                    nc.tensor.matmul(
                        h0_ps[:, sl], st["lw2"][:], st["rhs2"][:, sl],
                        start=True, stop=False,
                    )

            def emit_h0_pair(b, c, half, st):
                """fp8 DoubleRow k-pair matmul over dots strips (2c, 2c+1),
                one 512-wide j-half."""
                h0_ps, dots_sb = st["h0ps"], st["dots"]
                lhsT = w0d_sb[:, 256 * c: 256 * (c + 1)].rearrange(
                    "p (t n) -> p t n", t=2)
                pair = dots_sb[:, 2048 * c: 2048 * (c + 1)].rearrange(
                    "p (t h n) -> p t h n", t=2, h=2)
                nc.tensor.matmul(
                    h0_ps[:, 512 * half: 512 * (half + 1)],
                    lhsT,
                    pair[:, :, half, :],
                    start=False,
                    stop=(c == NSTRIP // 2 - 1),
                    perf_mode=DR,
                )

            def emit_h0_leaky(b, st):
                h0cA = actp.tile([128, 512], f16, tag="h0cA", name=f"h0cA{b}")
                h0cB = actp.tile([128, 512], f16, tag="h0cB", name=f"h0cB{b}")
                leaky_evict(h0cA, h0cB, st["h0ps"], on_act=(b == BPC - 1))
                st["h0c"] = (h0cA, h0cB)

            def emit_tail(b, st):
                """Layer 1 (swapped roles -> natural layout) + y matmuls,
                processed in j-halves flowing through separate half-tiles so
                PE/DVE/Act pipeline without false write-write ordering."""
                last = b == BPC - 1
                xn_sb = st["xn"]
                h1nh = (
                    actp.tile([128, 512], f16, tag="h1nA", name=f"h1nA{b}"),
                    actp.tile([128, 512], f16, tag="h1nB", name=f"h1nB{b}"),
                )
                y_ps = h0pp.tile([128, N], f32, tag="h0ps", name=f"yps{b}")
                for hi in range(2):
                    h1_ps = h1pp.tile([128, 512], f32, tag="h1ps",
                                      name=f"h1ps{b}_{hi}")
                    if with_b1:
                        nc.tensor.matmul(
                            h1_ps[:], ones1_sb[:],
                            b1t_sb[:, 512 * hi: 512 * (hi + 1)],
                            start=True, stop=False,
                        )
                    for t in range(4 * hi, 4 * hi + 4):
                        nc.tensor.matmul(
                            h1_ps[:, 128 * (t % 4): 128 * (t % 4 + 1)],
                            st["h0c"][t // 4][:, 128 * (t % 4): 128 * (t % 4 + 1)],
                            w1_sb[:],
                            start=not with_b1,
                            stop=True,
                        )
                    if last and hi == 1:
                        nc.scalar.activation(
                            h1nh[hi][:], h1_ps[:], PRELU,
                            bias=0.0, scale=1.0, alpha=0.01)
                    else:
                        nc.vector._custom_dve(
                            LEAKY, out=h1nh[hi][:], in0=h1_ps[:], s0=0.01)
                    for t in range(4 * hi, 4 * hi + 4):
                        nc.tensor.matmul(
                            y_ps[:, 0:3],
                            h1nh[hi][:, 128 * (t % 4): 128 * (t % 4 + 1)],
                            xn_sb[:, 3 * t: 3 * (t + 1)],
                            start=(t == 0),
                            stop=(t == NSTRIP - 1),
                        )
                y_sb = youtp.tile([128, 4], f32, tag="y", name=f"y{b}")
                nc.vector.tensor_copy(y_sb[:, 0:3], y_ps[:, 0:3])
                nc.sync.dma_start(out=y_d[b], in_=y_sb[:, 0:3])

            # Software-pipelined emission (half-symmetric). Strips 0-3 are
            # full rows; h0 pairs 0/1 depend only on them and run early as PE
            # fill. Strips 4-7: right halves computed directly (paired PSUM,
            # one sqrt per strip-pair), left halves reconstructed by PE
            # identity-transposes of strips 0-3's right-half tiles, evicted
            # by one DVE copy per strip-pair.
            def emit_all():
                states = [dict() for _ in range(BPC)]
                for b in range(BPC):
                    st = states[b]
                    emit_gram_strip(b, 0, st)
                    emit_gram_strip(b, 1, st)
                    emit_h0_init(b, st)
                    emit_h0_pair(b, 0, 0, st)
                    emit_h0_pair(b, 0, 1, st)
                    emit_gram_strip(b, 2, st)
                    emit_gram_strip(b, 3, st)
                    # right-half grams immediately: they only need xT, and
                    # ScalarE's sqrt stream must never wait for them.
                    emit_gram_rights(b, 4, st)
                    emit_gram_rights(b, 6, st)
                    emit_h0_pair(b, 1, 0, st)
                    emit_h0_pair(b, 1, 1, st)
                    if b >= 1:
                        emit_tail(b - 1, states[b - 1])
                    emit_slab(b, 4, st)
                    emit_evict(b, 4, st)
                    emit_slab(b, 5, st)
                    emit_evict(b, 5, st)
                    emit_slab(b, 6, st)
                    emit_evict(b, 6, st)
                    emit_slab(b, 7, st)
                    emit_evict(b, 7, st)
                    emit_h0_pair(b, 2, 0, st)
                    emit_h0_pair(b, 2, 1, st)
                    emit_h0_pair(b, 3, 0, st)
                    emit_h0_pair(b, 3, 1, st)
                    emit_h0_leaky(b, st)
                emit_tail(BPC - 1, states[BPC - 1])

            if repeat == 1:
                emit_all()
            else:
                with tc.For_i(0, repeat, 1):
                    emit_all()

    nc.finalize()
    return nc


def _host_prep(x, u, W0, b0, W1, b1):
    """Build per-core input maps."""
    f8 = ml_dtypes.float8_e4m3
    S = DOTS_SCALE
    xT = np.ascontiguousarray(x.transpose(0, 2, 1)).astype(np.float16)  # [B,3,N]
    xn = np.ascontiguousarray(
        x.reshape(B, NSTRIP, 128, 3).transpose(0, 2, 1, 3).reshape(B, 128, 3 * NSTRIP)
    ).astype(np.float16)
    norms = np.sqrt((x.astype(np.float64) ** 2).sum(-1)).astype(np.float32)  # [B,N]
    rhs2 = np.stack([norms, np.ones_like(norms)], axis=1).astype(np.float16)

    W0d = W0[G + 1:]                                    # [N, H]
    w0d8 = (W0d * S).astype(f8)                         # stored fp8
    w0d8_pe = np.ascontiguousarray(
        w0d8.reshape(NSTRIP, 128, H).transpose(1, 0, 2).reshape(128, NSTRIP * H)
    )
    # mean-field compensation for W0d quantization, using a sampled mu_dots
    rng = np.random.default_rng(12345)
    ii = rng.integers(0, N, size=8192)
    jj = rng.integers(0, N, size=8192)
    mu = np.sqrt(np.maximum((x[:, ii, :] * x[:, jj, :]).sum(-1), 0.0)).mean(axis=1)
    dW = W0d - w0d8.astype(np.float32) / S              # [N, H]
    cb = (u @ W0[:G] + b0).astype(np.float32)           # [B, H]
    cb = cb + mu[:, None] * dW.sum(axis=0)[None, :]
    w0n = np.broadcast_to(W0[G], (B, H)).astype(np.float32)
    lw2 = np.ascontiguousarray(np.stack([w0n, cb], axis=1)).astype(np.float16)

    in_maps = []
    for c in range(N_CORES):
        sl = slice(BPC * c, BPC * (c + 1))
        in_maps.append(
            {
                "xT": np.ascontiguousarray(xT[sl]),
                "xn": np.ascontiguousarray(xn[sl]),
                "rhs2": np.ascontiguousarray(rhs2[sl]),
                "lw2": np.ascontiguousarray(lw2[sl]),
                "w0d": w0d8_pe,
                "w1": np.ascontiguousarray(W1).astype(np.float16),
                "ident": np.eye(128, dtype=np.float32).astype(f8),
                "b1t": np.tile(b1, N // H)[None, :].astype(np.float16),
                "ones1": np.ones((1, 128), dtype=np.float16),
            }
        )
    return in_maps


def kernel(x, u, W0, b0, W1, b1, W2, b2, _run_kwargs=None):
    x = np.asarray(x, dtype=np.float32)
    u = np.asarray(u, dtype=np.float32)
    W0 = np.asarray(W0, dtype=np.float32)
    b0 = np.asarray(b0, dtype=np.float32)
    W1 = np.asarray(W1, dtype=np.float32)
    b1 = np.asarray(b1, dtype=np.float32)
    W2 = np.asarray(W2, dtype=np.float32)
    b2 = np.asarray(b2, dtype=np.float32)

    from concourse.bass_utils import run_bass_kernel_spmd

    with_b1 = bool(np.any(b1))
    key = ("nc", with_b1)
    if key not in _cached:
        _cached[key] = _build_nc(with_b1=with_b1)
    nc = _cached[key]

    in_maps = _host_prep(x, u, W0, b0, W1, b1)
    kw = dict(_run_kwargs or {})
    res = run_bass_kernel_spmd(nc, in_maps, list(range(N_CORES)), **kw)
    _cached["last_results"] = res
    y = np.concatenate([r["y"] for r in res.results], axis=0)  # [B, H, 3]

    # host finish: out[b,o,d] = sum_h W2[h,o] y[b,h,d] / N + b2[o]*colsum_x[b,d]/N
    colsum = x.sum(axis=1)  # [B, 3]
    out = (
        np.einsum("ho,bhd->bod", W2.astype(np.float64), y.astype(np.float64))
        + b2.astype(np.float64)[None, :, None] * colsum.astype(np.float64)[:, None, :]
    ) / N
    return out.astype(np.float32)
